# revision 67
# baseline (speedup 1.0000x reference)
"""Trainium2 Bass kernel for a single-layer MHA decode step with KV cache.

Problem (hardcoded from spec):
  x            [32, 8, 2048]      query tokens (B=32 batches x T=8 steps)
  cache_keys   [32, 32, 1016, 64] (B, H, S_cache, Dh)
  cache_values [32, 32, 1016, 64]
  Wq/Wk/Wv/Wo  [2048, 2048], biases [2048]
  out = MHA(x, cache) @ Wo.T + bo   -> [32, 8, 2048]

Sharding: tensor-parallel over heads. Each of the 8 cores handles 4 heads:
QKV projections for its head slice, attention over its KV-cache slice, and a
partial output projection (rank-256 slice of Wo). Host sums the 8 partials
and adds bo once.

Per-core design (DMA-bound at ~360 GB/s in the cost model):
 - K and V caches in fp8(e3m4); Wk/Wv in fp8 scaled by 128 (values ~0.02
   sit below e3m4's min normal); everything else fp16 (same bytes as bf16,
   ~8x less rounding noise).  All matmuls accumulate in fp32 PSUM.  The
   max-abs rel err measured on HW is 1.62e-2 against the 2e-2 gate.
 - Flipped matmul orientation: scores are computed keys-major
   (lhsT = K-chunk stationary, Q moving, out [128 keys, 32 toks]) so the PE
   streams only 32 token columns per matmul instead of 1024 key columns,
   and the attention matrix comes out already transposed for AV.  AV is
   flipped the same way (V stationary, attn moving, out [(h,d), toks]),
   writing attn-out^T directly in the Wo-ready layout -- no PE transposes.
 - Softmax: exp(s/8 - 6) with a constant shift (scores are O(5), fp32 exp
   cannot overflow).  Per-token sums via a ones-vector matmul on PE
   (partition reduction), reciprocal on DVE, broadcast back across
   partitions with a rank-1 ones matmul, staged to SBUF (HW allows one
   PSUM operand per DVE op); normalization is fused into the AV-psum
   evacuation.
 - Software pipelining: each round's body runs the previous round's
   V-phase (AV + softmax-denominator broadcast + normalized evacuation +
   output projection) so the PE's in-order round loop never waits on the
   current round's V arrivals or denominator chain; per-batch pss tiles
   avoid tile-granular WAR serialization of scores behind exp reads.
 - Key/score axis permuted by sigma(j) = 8*(j%128) + j//128, baked into kT
   on host.  Chunk c of the flipped score partitions is the stride-8 key set
   {8p + c}, matching the V pack layout.  Virtual s in [1016, 1024) (the
   freshly projected K/V) maps to kT columns 128c + 127 (zero-padded in the
   host pack, filled on-chip from the K projection); new V lands on
   partition 127 of the round's V tile via an SBUF->SBUF DMA.
 - Output projection is incremental and flipped (Wo chunk stationary, the
   round's 32 aoT tokens moving) producing a transposed staging tile; one
   deferred DMA covers rounds 0-6 in the post-stream idle window and round
   7 ships as a small two-half pipelined block on the tail.  The host
   un-transposes outT and sums the 8 partials.
 - DMA: one contiguous transfer per batch per tensor (>=1 KB per-partition
   descriptors, and few enough DMAs that HWDGE descriptor-gen at 625ns/DMA
   stays off the critical path); per round all K lands before all V, batch
   3 first, so the tail after the final transfer is only AV -> evac -> Wo.
"""

import numpy as np
import ml_dtypes

import concourse.bass as bass
import concourse.mybir as mybir
import concourse.tile as tile
from concourse import bacc
from concourse import bass_utils

F32 = mybir.dt.float32
F16 = mybir.dt.float16
FP8 = mybir.dt.float8e3

B, T, D = 32, 8, 2048
H, DH = 32, 64
S_CACHE, S = 1016, 1024
N_CORES = 8
HC = H // N_CORES          # heads per core = 4
TOK = B * T                # 256
QD = HC * DH               # 256 per-core qkv dims
N_ROUNDS = 8               # 4 batches per round
BB = 4                     # batches per round
NP = S_CACHE // 8          # 127 cache partitions in the V layout

AF = mybir.ActivationFunctionType
ALU = mybir.AluOpType

# Wk/Wv are ~N(0, 0.02) -- below fp8e3m4's 0.25 min normal -- so the host
# pre-scales them by WKV_SCALE and the PSUM evacuation divides it back out.
WKV_SCALE = 128.0

CFG = {"kt_hi": "fp16", "kt_lo": "fp8", "vcache": "fp8", "work": "fp16"}


def build_nc(cfg=CFG):
    nc = bacc.Bacc(None, target_bir_lowering=False)

    xT = nc.dram_tensor("xT", [128, 16, 256], F16, kind="ExternalInput")
    wqT = nc.dram_tensor("wqT", [128, 16, 256], F16, kind="ExternalInput")
    # Wk/Wv only shape the 8 fresh tokens (~1% of attention mass): fp8 is
    # harmless there and saves 2.9us of stream
    wkT = nc.dram_tensor("wkT", [128, 16, 256], FP8, kind="ExternalInput")
    wvT = nc.dram_tensor("wvT", [128, 16, 256], FP8, kind="ExternalInput")
    woT = nc.dram_tensor("woT", [128, 2, 2048], F16, kind="ExternalInput")
    bq = nc.dram_tensor("bq", [256], F32, kind="ExternalInput")
    bk = nc.dram_tensor("bk", [256], F32, kind="ExternalInput")
    bv = nc.dram_tensor("bv", [256], F32, kind="ExternalInput")
    # kT[p=(h,j), b, m, jcol]: sigma-permuted key columns (zeros at new-key
    # cols, filled on-chip from the K projection); fully fp8 (the max-err
    # impact measured on HW stays under the gate).  One DMA per batch keeps
    # the HWDGE descriptor-gen (625ns/DMA) off the stream's critical path.
    kT = nc.dram_tensor("kT", [128, B, 2, S], FP8, kind="ExternalInput")
    # v[p, b, i, h, d] = cache_values[b, h, 8p + i, d]
    v = nc.dram_tensor("v", [NP, B, 8, HC, DH], FP8, kind="ExternalInput")
    # transposed output: outT[p, r, ch, t] = out[32r + t, 128ch + p]
    outT = nc.dram_tensor("outT", [128, 8, 16, 32], F16,
                          kind="ExternalOutput")

    with tile.TileContext(nc) as tc:
        with (
            tc.tile_pool(name="singles", bufs=1) as singles,
            tc.tile_pool(name="stream", bufs=6) as stream,
            tc.tile_pool(name="attn_pool", bufs=2) as attn_pool,
            tc.tile_pool(name="small", bufs=4) as small,
            tc.tile_pool(name="ps_scores", bufs=1, space="PSUM") as ps_scores,
            tc.tile_pool(name="ps_round", bufs=2, space="PSUM") as ps_round,
            tc.tile_pool(name="ps_univ", bufs=2, space="PSUM") as ps_univ,
        ):
            # ---- persistent tiles ----
            xT_sb = singles.tile([128, 16, 256], F16)
            wq_sb = singles.tile([128, 16, 256], F16)
            wk_sb = singles.tile([128, 16, 256], FP8)
            wv_sb = singles.tile([128, 16, 256], FP8)
            wo_sb = singles.tile([128, 2, 2048], F16)
            # Q in block-diag layout: qbd[32h+j, m, 32b + 8h + t]
            qbd = singles.tile([128, 2, 1024], F16)
            nc.vector.memset(qbd, 0.0)

            nc.sync.dma_start(xT_sb, xT[:, :, :])
            nc.sync.dma_start(wq_sb, wqT[:, :, :])
            nc.sync.dma_start(wk_sb, wkT[:, :, :])

            def fetch_cache(r):
                """Allocate round r's K/V tiles.  All K before all V (batch
                3's K first): by the time the last V lands, every
                score/exp/sum chain is done, and the V-phase of round r runs
                software-pipelined inside round r+1's body."""
                kt = stream.tile([128, BB, 2, S], FP8, name=f"kt_{r}",
                                 tag="kt", bufs=6)
                vt = stream.tile([128, BB, 8, HC, DH], FP8,
                                 name=f"vt_{r}", tag="vt", bufs=6)
                for bb in (3, 0, 1, 2):
                    b = BB * r + bb
                    nc.sync.dma_start(kt[:, bb, :, :], kT[:, b, :, :])
                for bb in range(BB):
                    b = BB * r + bb
                    nc.sync.dma_start(vt[0:NP, bb, :, :, :], v[:, b, :, :, :])
                return kt, vt

            def fetch_vnew(r, vt):
                # freshly projected V onto partition 127 of the round's V tile
                nc.gpsimd.dma_start(
                    vt[NP:128, :, :, :, :],
                    vnew[r // 4][32 * (r % 4):32 * (r % 4) + 32, :])

            # wv ahead of the caches (V projection feeds round 0's new-V
            # row); wo up front too -- the incremental output projection
            # needs it from body 1, and a mid-stream load stalls the
            # in-order PE ~8.6us at wo_round(0)
            nc.sync.dma_start(wv_sb, wvT[:, :, :])
            nc.sync.dma_start(wo_sb, woT[:, :, :])
            tiles = {}
            for r in range(5):
                tiles[r] = fetch_cache(r)

            bq_sb = singles.tile([128, 2], F32)
            bk_sb = singles.tile([128, 2], F32)
            nc.gpsimd.dma_start(bq_sb, bq[:].rearrange("(m p) -> p m", p=128))
            nc.gpsimd.dma_start(bk_sb, bk[:].rearrange("(m p) -> p m", p=128))
            bv_bc = singles.tile([128, 256], F32)
            nc.gpsimd.dma_start(
                bv_bc, bass.AP(tensor=bv[:].tensor, offset=0, ap=[[0, 128], [1, 256]])
            )

            nbias_c = singles.tile([128, 1], F32)
            nc.vector.memset(nbias_c, -6.0)
            ones_c = singles.tile([128, 1], F16)
            nc.vector.memset(ones_c, 1.0)
            ones_r = singles.tile([1, 128], F32)
            nc.vector.memset(ones_r, 1.0)
            knew = singles.tile([128, 2, 256], F16)  # [q=(h,j), m, (b,t)]
            # attnout^T accumulated: [ao-half p, a, tok]
            aoT = singles.tile([128, 2, 256], F16)
            # freshly projected V, row (8 b_local + t), col (64 h + d)
            vnew = [singles.tile([128, 256], FP8, name=f"vnew_{m}")
                    for m in range(2)]
            # staged transposed output, [p, r, (ch, t)]
            osbT = singles.tile([128, 8, 512], F16, name="osbT")

            # ---- PE warm-up: junk matmuls bridge the p-state ramp so the
            # projections (and everything after) run at full clock.  Reads
            # the zeroed qbd tile; result discarded. ----
            ps_warm = ps_univ.tile([128, 512], F32, name="ps_warm",
                                   tag="u")[:, :128]
            for _ in range(100):
                nc.tensor.matmul(ps_warm, qbd[:, 0, 0:128], qbd[:, 0, 0:128],
                                 start=True, stop=True)

            # ---- projections ----
            for m in range(2):
                psq = ps_univ.tile([128, 512], F32, name=f"psq_{m}", tag="u")[:, :256]
                psk = ps_univ.tile([128, 512], F32, name=f"psk_{m}", tag="u")[:, :256]
                for k in range(16):
                    st = dict(start=(k == 0), stop=(k == 15))
                    nc.tensor.matmul(
                        psq, wq_sb[:, k, 128 * m:128 * m + 128],
                        xT_sb[:, k, :], **st)
                for k in range(16):
                    st = dict(start=(k == 0), stop=(k == 15))
                    nc.tensor.matmul(
                        psk, wk_sb[:, k, 128 * m:128 * m + 128],
                        xT_sb[:, k, :], **st)
                # evac Q into block-diag (strided) + bias; psum rows 32h+j
                for h in range(4):
                    rows = slice(32 * h, 32 * h + 32)
                    out_ap = qbd[rows, m, :].rearrange("p (b w) -> p b w", w=32)[
                        :, :, 8 * h:8 * h + 8
                    ]
                    in_ap = psq[rows, :].rearrange("p (b t) -> p b t", t=8)
                    nc.scalar.activation(out_ap, in_ap, AF.Identity,
                                         bias=bq_sb[rows, m:m + 1], scale=1.0)
                nc.scalar.activation(knew[:, m, :], psk, AF.Identity,
                                     bias=bk_sb[:, m:m + 1], scale=1.0 / WKV_SCALE)

            for m in range(2):
                psv = ps_univ.tile([128, 512], F32, name=f"psv_{m}", tag="u")[:, :256]
                for k in range(16):
                    st = dict(start=(k == 0), stop=(k == 15))
                    nc.tensor.matmul(
                        psv, xT_sb[:, k, 128 * m:128 * m + 128],
                        wv_sb[:, k, :], **st)
                nc.vector.scalar_tensor_tensor(
                    vnew[m], psv, 1.0 / WKV_SCALE, bv_bc,
                    ALU.mult, ALU.add)

            def fill_knew(r, kt):
                # new-K scores live at kt columns j = 128c + 127; per-batch
                # copies on the (otherwise idle) gpsimd engine so each scores
                # group waits only on its own kt chunk.
                for bb in (3, 0, 1, 2):   # batch 3's kt lands first
                    b = BB * r + bb
                    for m in range(2):
                        dst = kt[:, bb, m, :].rearrange(
                            "p (c w) -> p c w", w=128)[:, :, 127]
                        nc.gpsimd.tensor_copy(dst, knew[:, m, 8 * b:8 * b + 8])

            for r in range(5):
                fetch_vnew(r, tiles[r][1])
                fill_knew(r, tiles[r][0])

            def wo_round(r):
                # flipped output projection for round r's 32 tokens: Wo
                # chunk stationary, aoT tokens moving (32-col matmuls);
                # result is transposed [dcol, tok] and staged in osbT.
                # Round 7 runs in two pipelined halves (mm -> evac -> DMA)
                # so the tail overlaps its own stages.
                if r != 7:
                    psoT = ps_univ.tile([128, 512], F32, name=f"psoT_{r}",
                                        tag="u")
                    for ch in range(16):
                        for a in range(2):
                            nc.tensor.matmul(
                                psoT[:, 32 * ch:32 * ch + 32],
                                wo_sb[:, a, 128 * ch:128 * ch + 128],
                                aoT[:, a, 32 * r:32 * r + 32],
                                start=(a == 0), stop=(a == 1))
                    nc.vector.tensor_copy(osbT[:, r, :], psoT)
                    return
                # round 7 (the tail): two SEPARATE psum tiles so half 1's
                # matmuls don't WAR-block on half 0's evacuation read, with
                # the evac copies pipelined behind the matmuls
                halves = [ps_univ.tile([128, 256], F32, name=f"psoT_7{h}",
                                       tag="u") for h in range(2)]
                for half in range(2):
                    for ch in range(8 * half, 8 * half + 8):
                        for a in range(2):
                            nc.tensor.matmul(
                                halves[half][:, 32 * (ch % 8):32 * (ch % 8) + 32],
                                wo_sb[:, a, 128 * ch:128 * ch + 128],
                                aoT[:, a, 32 * r:32 * r + 32],
                                start=(a == 0), stop=(a == 1))
                for half in range(2):
                    cols = slice(256 * half, 256 * half + 256)
                    nc.vector.tensor_copy(osbT[:, 7, cols], halves[half])
                nc.sync.dma_start(
                    outT[:, 7, :, :],
                    osbT[:, 7, :].rearrange("p (ch t) -> p ch t", t=32))

            def av_part1(r, vt, attnE, pav, psbc, recip):
                # AV matmuls of round r, software-pipelined into round r+1's
                # body: by then all of round r's V has landed, so nothing
                # here stalls the PE (the tail pays only for round 7).
                for bb in range(BB):
                    for hp in range(2):
                        lhsT = vt[:, bb, :, 2 * hp:2 * hp + 2, :]
                        for c in range(8):
                            nc.tensor.matmul(
                                pav[:, bb, hp, :],
                                lhsT[:, c, :, :].rearrange("p a d -> p (a d)"),
                                attnE[:, c, bb, 16 * hp:16 * hp + 16],
                                start=(c == 0), stop=(c == 7))

            def psbc_rb(r, psbc, recip):
                # denominator broadcast for round r (recip(r) already done)
                nc.tensor.matmul(psbc, ones_r[0:1, :], recip[0:1, :],
                                 start=True, stop=True)
                # HW allows only one PSUM operand per DVE op: stage the
                # broadcast reciprocals in SBUF for the evac multiply.
                # On DVE: an ACT copy would sit between rounds' exps on the
                # in-order ACT queue, chaining the denominator path into
                # every round.
                rb = small.tile([128, 128], F32, name=f"rb_{r}", tag="rb",
                                bufs=2)
                nc.vector.tensor_copy(rb, psbc)
                return rb

            def evacs(r, pav, rb):
                # normalized evacuation; must be issued after the AVs
                # (program-order deps) and after psbc_rb
                rbv = rb.rearrange("p (b h e t) -> p b h e t", b=4, h=2, e=2)
                for e in range(2):
                    rows = slice(64 * e, 64 * e + 64)
                    dst = aoT[rows, :, 32 * r:32 * r + 32].rearrange(
                        "p a (b t) -> p a b t", t=8)
                    src = pav[rows, :, :, 8 * e:8 * e + 8].rearrange(
                        "p b h t -> p h b t")
                    rbe = rbv[rows, :, :, e, :].rearrange("p b h t -> p h b t")
                    nc.vector.tensor_mul(dst, src, rbe)

            def evac_chain(r, pav, psbc, recip):
                # (in body r+1, where recip(r) is already done -- keeping
                # the DVE reciprocal hop out of the PE's round loop)
                rb = psbc_rb(r, psbc, recip)
                evacs(r, pav, rb)

            prev = None
            for r in range(N_ROUNDS):
                kt, vt = tiles.pop(r)
                if prev is not None:
                    av_part1(*prev)
                    evac_chain(prev[0], prev[3], prev[4], prev[5])
                # ---- K-phase: scores -> exp (per batch), then sums ----
                # one pss tile PER BATCH: tile deps are WAR-coupled at tile
                # granularity, so a shared tile would serialize each batch's
                # scores behind the previous batch's exp read
                attnE = attn_pool.tile([128, 8, 4, 32], F16, name=f"attnE_{r}",
                                       tag="attn")
                pr = ps_round.tile([128, 512], F32, name=f"pr_{r}", tag="pr")
                pav = pr[:, 0:128].rearrange("p (b h w) -> p b h w", b=4, h=2)
                psbc = pr[:, 128:256]
                # all scores first, then all sums: interleaving them makes a
                # PE<->ACT ladder (sums(bb) stalls PE on exp(bb)) that delays
                # the last batch ~2us past its kt arrival
                for bb in (3, 0, 1, 2):   # batch 3's kt lands first
                    b = BB * r + bb
                    pss = ps_scores.tile([128, 8, 32], F32,
                                         name=f"pss_{r}_{bb}",
                                         tag=f"pss{bb}", bufs=1)
                    for c in range(8):
                        for m in range(2):
                            nc.tensor.matmul(
                                pss[:, c, :],
                                kt[:, bb, m, 128 * c:128 * c + 128],
                                qbd[:, m, 32 * b:32 * b + 32],
                                start=(m == 0), stop=(m == 1))
                    nc.scalar.activation(attnE[:, :, bb, :], pss,
                                         AF.Exp, bias=nbias_c, scale=0.125)
                for bb in (3, 0, 1, 2):
                    psum_b = pr[0:1, 256 + 32 * bb:256 + 32 * bb + 32]
                    for c in range(8):
                        nc.tensor.matmul(psum_b, ones_c[:, 0:1],
                                         attnE[:, c, bb, :],
                                         start=(c == 0), stop=(c == 7))
                if r + 5 < N_ROUNDS:
                    t5 = fetch_cache(r + 5)
                    fetch_vnew(r + 5, t5[1])
                    tiles[r + 5] = t5
                    fill_knew(r + 5, t5[0])
                # ---- softmax denominators (broadcast happens in av_part1)
                recip = small.tile([1, 128], F32, name=f"recip_{r}",
                                   tag="recip", bufs=2)
                nc.vector.reciprocal(recip, pr[0:1, 256:384])
                if prev is not None:
                    wo_round(prev[0])
                prev = (r, vt, attnE, pav, psbc, recip)
            # rounds 0-6 output DMA first: its transfer fills the
            # post-stream idle window while round 7's V-phase finishes
            nc.sync.dma_start(
                outT[:, 0:7, :, :],
                osbT[:, 0:7, :].rearrange("p r (ch t) -> p r ch t", t=32))
            av_part1(*prev)
            evac_chain(prev[0], prev[3], prev[4], prev[5])
            wo_round(prev[0])

    nc.finalize()
    return nc


_SIGMA = None


def _sigma():
    # sigma(j) = virtual key index at score column j
    global _SIGMA
    if _SIGMA is None:
        j = np.arange(S)
        _SIGMA = 8 * (j % 128) + j // 128
    return _SIGMA


F16_NP = np.float16
FP8_NP = ml_dtypes.float8_e3m4


def _prep_core(c, x_flat_T, cache_keys, cache_values,
               Wq, bq, Wk, bk, Wv, bv, Wo, bo=None, cfg=CFG):
    hs = slice(HC * c, HC * c + HC)
    qs = slice(QD * c, QD * c + QD)

    def perm_rows(W):
        # rows ordered (m, h, j): row 32h + j of tile m = W[64h + 32m + j]
        Ws = W[qs].reshape(HC, 2, 32, -1)              # [h, m, j, d]
        return Ws.transpose(1, 0, 2, 3).reshape(QD, -1)  # [(m,h,j), d]

    wq_p = perm_rows(Wq)
    wk_p = perm_rows(Wk)
    bq_p = np.ascontiguousarray(perm_rows(bq[:, None])[:, 0])
    bk_p = np.ascontiguousarray(perm_rows(bk[:, None])[:, 0])

    def as_tiles(WT):  # [D, 256] -> [128, 16, 256]
        return np.ascontiguousarray(
            WT.reshape(16, 128, QD).transpose(1, 0, 2)).astype(F16_NP)

    wqT = as_tiles(np.ascontiguousarray(wq_p.T))
    wkT = (as_tiles(np.ascontiguousarray(wk_p.T)).astype(np.float32)
           * WKV_SCALE).astype(FP8_NP)
    wvT = (as_tiles(np.ascontiguousarray(Wv[qs].T)).astype(np.float32)
           * WKV_SCALE).astype(FP8_NP)
    woT = np.ascontiguousarray(
        Wo[:, qs].T.reshape(2, 128, D).transpose(1, 0, 2)).astype(F16_NP)

    # kT[p=(h,j), b, m, jcol]: keys sigma-permuted; zero at new-key columns
    ck = cache_keys[:, hs]                        # [B, 4, 1016, 64]
    kmat = ck.reshape(B, HC, S_CACHE, 2, 32).transpose(0, 3, 1, 4, 2)  # b m h j s
    kmat = np.ascontiguousarray(kmat.reshape(B, 2, 128, S_CACHE))
    kT = np.zeros((B, 2, 128, S), dtype=np.float32)
    sig = _sigma()
    valid = sig < S_CACHE
    kT[:, :, :, valid] = kmat[:, :, :, sig[valid]]
    kT = np.ascontiguousarray(kT.transpose(2, 0, 1, 3)).astype(FP8_NP)

    # v[p, b, i, h, d] = cache_values[b, h, 8p + i, d]
    cv = cache_values[:, hs].reshape(B, HC, NP, 8, DH)
    v_pack = np.ascontiguousarray(cv.transpose(2, 0, 3, 1, 4)).astype(FP8_NP)

    return {
        "xT": x_flat_T.astype(F16_NP),
        "wqT": wqT, "wkT": wkT, "wvT": wvT, "woT": woT,
        "bq": bq_p, "bk": bk_p,
        "bv": np.ascontiguousarray(bv[qs]),
        "kT": kT,
        "v": v_pack,
    }


def merge_outputs(outT_arr):
    """Un-transpose a core's partial: outT[p, r, ch, t] -> [256, 2048]."""
    tl = np.asarray(outT_arr, dtype=np.float32)  # [128, 8, 16, 32]
    return tl.transpose(1, 3, 2, 0).reshape(TOK, D)


_NC_CACHE = {}


def kernel(x, cache_keys, cache_values, Wq, bq, Wk, bk, Wv, bv, Wo, bo):
    x = np.asarray(x, dtype=np.float32)
    cache_keys = np.asarray(cache_keys, dtype=np.float32)
    cache_values = np.asarray(cache_values, dtype=np.float32)
    Wq, Wk, Wv, Wo = (np.asarray(w, dtype=np.float32) for w in (Wq, Wk, Wv, Wo))
    bq, bk, bv, bo = (np.asarray(b_, dtype=np.float32) for b_ in (bq, bk, bv, bo))

    x_flat_T = np.ascontiguousarray(
        x.reshape(TOK, D).T.reshape(16, 128, TOK).transpose(1, 0, 2))  # [128,16,256]

    in_maps = [
        _prep_core(c, x_flat_T, cache_keys, cache_values,
                   Wq, bq, Wk, bk, Wv, bv, Wo)
        for c in range(N_CORES)
    ]

    key = tuple(sorted(CFG.items()))
    if key not in _NC_CACHE:
        _NC_CACHE[key] = build_nc(CFG)
    nc = _NC_CACHE[key]

    res = bass_utils.run_bass_kernel_spmd(nc, in_maps, core_ids=list(range(N_CORES)))
    out = np.zeros((TOK, D), dtype=np.float32)
    for r in res.results:
        out += merge_outputs(r["outT"])
    out += bo
    return out.reshape(B, T, D)


# revision 70
# speedup vs baseline: 1.0011x; 1.0011x over previous
"""Trainium2 Bass kernel for a single-layer MHA decode step with KV cache.

Problem (hardcoded from spec):
  x            [32, 8, 2048]      query tokens (B=32 batches x T=8 steps)
  cache_keys   [32, 32, 1016, 64] (B, H, S_cache, Dh)
  cache_values [32, 32, 1016, 64]
  Wq/Wk/Wv/Wo  [2048, 2048], biases [2048]
  out = MHA(x, cache) @ Wo.T + bo   -> [32, 8, 2048]

Sharding: tensor-parallel over heads. Each of the 8 cores handles 4 heads:
QKV projections for its head slice, attention over its KV-cache slice, and a
partial output projection (rank-256 slice of Wo). Host sums the 8 partials
and adds bo once.

Per-core design (DMA-bound at ~360 GB/s in the cost model):
 - K and V caches in fp8(e3m4); Wk/Wv in fp8 scaled by 128 (values ~0.02
   sit below e3m4's min normal); everything else fp16 (same bytes as bf16,
   ~8x less rounding noise).  All matmuls accumulate in fp32 PSUM.  The
   max-abs rel err measured on HW is 1.62e-2 against the 2e-2 gate.
 - Flipped matmul orientation: scores are computed keys-major
   (lhsT = K-chunk stationary, Q moving, out [128 keys, 32 toks]) so the PE
   streams only 32 token columns per matmul instead of 1024 key columns,
   and the attention matrix comes out already transposed for AV.  AV is
   flipped the same way (V stationary, attn moving, out [(h,d), toks]),
   writing attn-out^T directly in the Wo-ready layout -- no PE transposes.
 - Softmax: exp(s/8 - 6) with a constant shift (scores are O(5), fp32 exp
   cannot overflow).  Per-token sums via a ones-vector matmul on PE
   (partition reduction), reciprocal on DVE, broadcast back across
   partitions with a rank-1 ones matmul, staged to SBUF (HW allows one
   PSUM operand per DVE op); normalization is fused into the AV-psum
   evacuation.
 - Software pipelining: each round's body runs the previous round's
   V-phase (AV + softmax-denominator broadcast + normalized evacuation +
   output projection) so the PE's in-order round loop never waits on the
   current round's V arrivals or denominator chain; per-batch pss tiles
   avoid tile-granular WAR serialization of scores behind exp reads.
 - Key/score axis permuted by sigma(j) = 8*(j%128) + j//128, baked into kT
   on host.  Chunk c of the flipped score partitions is the stride-8 key set
   {8p + c}, matching the V pack layout.  Virtual s in [1016, 1024) (the
   freshly projected K/V) maps to kT columns 128c + 127 (zero-padded in the
   host pack, filled on-chip from the K projection); new V lands on
   partition 127 of the round's V tile via an SBUF->SBUF DMA.
 - Output projection is incremental and flipped (Wo chunk stationary, the
   round's 32 aoT tokens moving) producing a transposed staging tile; one
   deferred DMA covers rounds 0-6 in the post-stream idle window and round
   7 ships as a small two-half pipelined block on the tail.  The host
   un-transposes outT and sums the 8 partials.
 - DMA: one contiguous transfer per batch per tensor (>=1 KB per-partition
   descriptors, and few enough DMAs that HWDGE descriptor-gen at 625ns/DMA
   stays off the critical path); per round all K lands before all V, batch
   3 first, so the tail after the final transfer is only AV -> evac -> Wo.
"""

import numpy as np
import ml_dtypes

import concourse.bass as bass
import concourse.mybir as mybir
import concourse.tile as tile
from concourse import bacc
from concourse import bass_utils

F32 = mybir.dt.float32
F16 = mybir.dt.float16
FP8 = mybir.dt.float8e3

B, T, D = 32, 8, 2048
H, DH = 32, 64
S_CACHE, S = 1016, 1024
N_CORES = 8
HC = H // N_CORES          # heads per core = 4
TOK = B * T                # 256
QD = HC * DH               # 256 per-core qkv dims
N_ROUNDS = 8               # 4 batches per round
BB = 4                     # batches per round
NP = S_CACHE // 8          # 127 cache partitions in the V layout

AF = mybir.ActivationFunctionType
ALU = mybir.AluOpType

# Wk/Wv are ~N(0, 0.02) -- below fp8e3m4's 0.25 min normal -- so the host
# pre-scales them by WKV_SCALE and the PSUM evacuation divides it back out.
WKV_SCALE = 128.0

CFG = {"kt_hi": "fp16", "kt_lo": "fp8", "vcache": "fp8", "work": "fp16"}


def build_nc(cfg=CFG):
    nc = bacc.Bacc(None, target_bir_lowering=False)

    xT = nc.dram_tensor("xT", [128, 16, 256], F16, kind="ExternalInput")
    wqT = nc.dram_tensor("wqT", [128, 16, 256], F16, kind="ExternalInput")
    # Wk/Wv only shape the 8 fresh tokens (~1% of attention mass): fp8 is
    # harmless there and saves 2.9us of stream
    wkT = nc.dram_tensor("wkT", [128, 16, 256], FP8, kind="ExternalInput")
    wvT = nc.dram_tensor("wvT", [128, 16, 256], FP8, kind="ExternalInput")
    woT = nc.dram_tensor("woT", [128, 2, 2048], F16, kind="ExternalInput")
    bq = nc.dram_tensor("bq", [256], F32, kind="ExternalInput")
    bk = nc.dram_tensor("bk", [256], F32, kind="ExternalInput")
    bv = nc.dram_tensor("bv", [256], F32, kind="ExternalInput")
    # kT[p=(h,j), b, m, jcol]: sigma-permuted key columns (zeros at new-key
    # cols, filled on-chip from the K projection); fully fp8 (the max-err
    # impact measured on HW stays under the gate).  One DMA per batch keeps
    # the HWDGE descriptor-gen (625ns/DMA) off the stream's critical path.
    kT = nc.dram_tensor("kT", [128, B, 2, S], FP8, kind="ExternalInput")
    # v[p, b, i, h, d] = cache_values[b, h, 8p + i, d]
    v = nc.dram_tensor("v", [NP, B, 8, HC, DH], FP8, kind="ExternalInput")
    # transposed output: outT[p, r, ch, t] = out[32r + t, 128ch + p]
    outT = nc.dram_tensor("outT", [128, 8, 16, 32], F16,
                          kind="ExternalOutput")

    with tile.TileContext(nc) as tc:
        with (
            tc.tile_pool(name="singles", bufs=1) as singles,
            tc.tile_pool(name="stream", bufs=6) as stream,
            tc.tile_pool(name="attn_pool", bufs=3) as attn_pool,
            tc.tile_pool(name="small", bufs=4) as small,
            tc.tile_pool(name="ps_scores", bufs=1, space="PSUM") as ps_scores,
            tc.tile_pool(name="ps_round", bufs=2, space="PSUM") as ps_round,
            tc.tile_pool(name="ps_univ", bufs=2, space="PSUM") as ps_univ,
        ):
            # ---- persistent tiles ----
            xT_sb = singles.tile([128, 16, 256], F16)
            wq_sb = singles.tile([128, 16, 256], F16)
            wk_sb = singles.tile([128, 16, 256], FP8)
            wv_sb = singles.tile([128, 16, 256], FP8)
            wo_sb = singles.tile([128, 2, 2048], F16)
            # Q in block-diag layout: qbd[32h+j, m, 32b + 8h + t]
            qbd = singles.tile([128, 2, 1024], F16)
            nc.vector.memset(qbd, 0.0)

            nc.sync.dma_start(xT_sb, xT[:, :, :])
            nc.sync.dma_start(wq_sb, wqT[:, :, :])
            nc.sync.dma_start(wk_sb, wkT[:, :, :])

            def fetch_cache(r):
                """Allocate round r's K/V tiles.  All K before all V (batch
                3's K first): by the time the last V lands, every
                score/exp/sum chain is done, and the V-phase of round r runs
                software-pipelined inside round r+1's body."""
                kt = stream.tile([128, BB, 2, S], FP8, name=f"kt_{r}",
                                 tag="kt", bufs=6)
                vt = stream.tile([128, BB, 8, HC, DH], FP8,
                                 name=f"vt_{r}", tag="vt", bufs=6)
                for bb in (3, 0, 1, 2):
                    b = BB * r + bb
                    nc.sync.dma_start(kt[:, bb, :, :], kT[:, b, :, :])
                for bb in range(BB):
                    b = BB * r + bb
                    nc.sync.dma_start(vt[0:NP, bb, :, :, :], v[:, b, :, :, :])
                return kt, vt

            def fetch_vnew(r, vt):
                # freshly projected V onto partition 127 of the round's V tile
                nc.gpsimd.dma_start(
                    vt[NP:128, :, :, :, :],
                    vnew[r // 4][32 * (r % 4):32 * (r % 4) + 32, :])

            # wv ahead of the caches (V projection feeds round 0's new-V
            # row); wo up front too -- the incremental output projection
            # needs it from body 1, and a mid-stream load stalls the
            # in-order PE ~8.6us at wo_round(0)
            nc.sync.dma_start(wv_sb, wvT[:, :, :])
            nc.sync.dma_start(wo_sb, woT[:, :, :])
            tiles = {}
            for r in range(5):
                tiles[r] = fetch_cache(r)

            bq_sb = singles.tile([128, 2], F32)
            bk_sb = singles.tile([128, 2], F32)
            nc.gpsimd.dma_start(bq_sb, bq[:].rearrange("(m p) -> p m", p=128))
            nc.gpsimd.dma_start(bk_sb, bk[:].rearrange("(m p) -> p m", p=128))
            bv_bc = singles.tile([128, 256], F32)
            nc.gpsimd.dma_start(
                bv_bc, bass.AP(tensor=bv[:].tensor, offset=0, ap=[[0, 128], [1, 256]])
            )

            nbias_c = singles.tile([128, 1], F32)
            nc.vector.memset(nbias_c, -6.0)
            ones_c = singles.tile([128, 1], F16)
            nc.vector.memset(ones_c, 1.0)
            ones_r = singles.tile([1, 128], F32)
            nc.vector.memset(ones_r, 1.0)
            knew = singles.tile([128, 2, 256], F16)  # [q=(h,j), m, (b,t)]
            # attnout^T accumulated: [ao-half p, a, tok]
            aoT = singles.tile([128, 2, 256], F16)
            # freshly projected V, row (8 b_local + t), col (64 h + d)
            vnew = [singles.tile([128, 256], FP8, name=f"vnew_{m}")
                    for m in range(2)]
            # staged transposed output, [p, r, (ch, t)]
            osbT = singles.tile([128, 8, 512], F16, name="osbT")

            # ---- PE warm-up: junk matmuls bridge the p-state ramp so the
            # projections (and everything after) run at full clock.  Reads
            # the zeroed qbd tile; result discarded. ----
            ps_warm = ps_univ.tile([128, 512], F32, name="ps_warm",
                                   tag="u")[:, :128]
            for _ in range(100):
                nc.tensor.matmul(ps_warm, qbd[:, 0, 0:128], qbd[:, 0, 0:128],
                                 start=True, stop=True)

            # ---- projections ----
            for m in range(2):
                psq = ps_univ.tile([128, 512], F32, name=f"psq_{m}", tag="u")[:, :256]
                psk = ps_univ.tile([128, 512], F32, name=f"psk_{m}", tag="u")[:, :256]
                for k in range(16):
                    st = dict(start=(k == 0), stop=(k == 15))
                    nc.tensor.matmul(
                        psq, wq_sb[:, k, 128 * m:128 * m + 128],
                        xT_sb[:, k, :], **st)
                for k in range(16):
                    st = dict(start=(k == 0), stop=(k == 15))
                    nc.tensor.matmul(
                        psk, wk_sb[:, k, 128 * m:128 * m + 128],
                        xT_sb[:, k, :], **st)
                # evac Q into block-diag (strided) + bias; psum rows 32h+j
                for h in range(4):
                    rows = slice(32 * h, 32 * h + 32)
                    out_ap = qbd[rows, m, :].rearrange("p (b w) -> p b w", w=32)[
                        :, :, 8 * h:8 * h + 8
                    ]
                    in_ap = psq[rows, :].rearrange("p (b t) -> p b t", t=8)
                    nc.scalar.activation(out_ap, in_ap, AF.Identity,
                                         bias=bq_sb[rows, m:m + 1], scale=1.0)
                nc.scalar.activation(knew[:, m, :], psk, AF.Identity,
                                     bias=bk_sb[:, m:m + 1], scale=1.0 / WKV_SCALE)

            for m in range(2):
                psv = ps_univ.tile([128, 512], F32, name=f"psv_{m}", tag="u")[:, :256]
                for k in range(16):
                    st = dict(start=(k == 0), stop=(k == 15))
                    nc.tensor.matmul(
                        psv, xT_sb[:, k, 128 * m:128 * m + 128],
                        wv_sb[:, k, :], **st)
                nc.vector.scalar_tensor_tensor(
                    vnew[m], psv, 1.0 / WKV_SCALE, bv_bc,
                    ALU.mult, ALU.add)

            def fill_knew(r, kt):
                # new-K scores live at kt columns j = 128c + 127; per-batch
                # copies on the (otherwise idle) gpsimd engine so each scores
                # group waits only on its own kt chunk.
                for bb in (3, 0, 1, 2):   # batch 3's kt lands first
                    b = BB * r + bb
                    for m in range(2):
                        dst = kt[:, bb, m, :].rearrange(
                            "p (c w) -> p c w", w=128)[:, :, 127]
                        nc.gpsimd.tensor_copy(dst, knew[:, m, 8 * b:8 * b + 8])

            for r in range(5):
                fetch_vnew(r, tiles[r][1])
                fill_knew(r, tiles[r][0])

            def wo_round(r):
                # flipped output projection for round r's 32 tokens: Wo
                # chunk stationary, aoT tokens moving (32-col matmuls);
                # result is transposed [dcol, tok] and staged in osbT.
                # Round 7 runs in two pipelined halves (mm -> evac -> DMA)
                # so the tail overlaps its own stages.
                if r != 7:
                    psoT = ps_univ.tile([128, 512], F32, name=f"psoT_{r}",
                                        tag="u")
                    for ch in range(16):
                        for a in range(2):
                            nc.tensor.matmul(
                                psoT[:, 32 * ch:32 * ch + 32],
                                wo_sb[:, a, 128 * ch:128 * ch + 128],
                                aoT[:, a, 32 * r:32 * r + 32],
                                start=(a == 0), stop=(a == 1))
                    nc.vector.tensor_copy(osbT[:, r, :], psoT)
                    return
                # round 7 (the tail): two SEPARATE psum tiles so half 1's
                # matmuls don't WAR-block on half 0's evacuation read, with
                # the evac copies pipelined behind the matmuls
                halves = [ps_univ.tile([128, 256], F32, name=f"psoT_7{h}",
                                       tag="u") for h in range(2)]
                for half in range(2):
                    for ch in range(8 * half, 8 * half + 8):
                        for a in range(2):
                            nc.tensor.matmul(
                                halves[half][:, 32 * (ch % 8):32 * (ch % 8) + 32],
                                wo_sb[:, a, 128 * ch:128 * ch + 128],
                                aoT[:, a, 32 * r:32 * r + 32],
                                start=(a == 0), stop=(a == 1))
                for half in range(2):
                    cols = slice(256 * half, 256 * half + 256)
                    nc.vector.tensor_copy(osbT[:, 7, cols], halves[half])
                nc.sync.dma_start(
                    outT[:, 7, :, :],
                    osbT[:, 7, :].rearrange("p (ch t) -> p ch t", t=32))

            def av_part1(r, vt, attnE, pav, psbc, recip):
                # AV matmuls of round r, software-pipelined into round r+1's
                # body: by then all of round r's V has landed, so nothing
                # here stalls the PE (the tail pays only for round 7).
                for bb in range(BB):
                    for hp in range(2):
                        lhsT = vt[:, bb, :, 2 * hp:2 * hp + 2, :]
                        for c in range(8):
                            nc.tensor.matmul(
                                pav[:, bb, hp, :],
                                lhsT[:, c, :, :].rearrange("p a d -> p (a d)"),
                                attnE[:, c, bb, 16 * hp:16 * hp + 16],
                                start=(c == 0), stop=(c == 7))

            def psbc_rb(r, psbc, recip):
                # denominator broadcast for round r (recip(r) already done)
                nc.tensor.matmul(psbc, ones_r[0:1, :], recip[0:1, :],
                                 start=True, stop=True)
                # HW allows only one PSUM operand per DVE op: stage the
                # broadcast reciprocals in SBUF for the evac multiply.
                # On DVE: an ACT copy would sit between rounds' exps on the
                # in-order ACT queue, chaining the denominator path into
                # every round.
                rb = small.tile([128, 128], F32, name=f"rb_{r}", tag="rb",
                                bufs=2)
                nc.vector.tensor_copy(rb, psbc)
                return rb

            def evacs(r, pav, rb):
                # normalized evacuation; must be issued after the AVs
                # (program-order deps) and after psbc_rb
                rbv = rb.rearrange("p (b h e t) -> p b h e t", b=4, h=2, e=2)
                for e in range(2):
                    rows = slice(64 * e, 64 * e + 64)
                    dst = aoT[rows, :, 32 * r:32 * r + 32].rearrange(
                        "p a (b t) -> p a b t", t=8)
                    src = pav[rows, :, :, 8 * e:8 * e + 8].rearrange(
                        "p b h t -> p h b t")
                    rbe = rbv[rows, :, :, e, :].rearrange("p b h t -> p h b t")
                    nc.vector.tensor_mul(dst, src, rbe)

            def evac_chain(r, pav, psbc, recip):
                # (in body r+1, where recip(r) is already done -- keeping
                # the DVE reciprocal hop out of the PE's round loop)
                rb = psbc_rb(r, psbc, recip)
                evacs(r, pav, rb)

            prev = None
            for r in range(N_ROUNDS):
                kt, vt = tiles.pop(r)
                if prev is not None:
                    av_part1(*prev)
                    evac_chain(prev[0], prev[3], prev[4], prev[5])
                # ---- K-phase: scores -> exp (per batch), then sums ----
                # one pss tile PER BATCH: tile deps are WAR-coupled at tile
                # granularity, so a shared tile would serialize each batch's
                # scores behind the previous batch's exp read
                attnE = attn_pool.tile([128, 8, 4, 32], F16, name=f"attnE_{r}",
                                       tag="attn")
                pr = ps_round.tile([128, 512], F32, name=f"pr_{r}", tag="pr")
                pav = pr[:, 0:128].rearrange("p (b h w) -> p b h w", b=4, h=2)
                psbc = pr[:, 128:256]
                # all scores first, then all sums: interleaving them makes a
                # PE<->ACT ladder (sums(bb) stalls PE on exp(bb)) that delays
                # the last batch ~2us past its kt arrival
                for bb in (3, 0, 1, 2):   # batch 3's kt lands first
                    b = BB * r + bb
                    pss = ps_scores.tile([128, 8, 32], F32,
                                         name=f"pss_{r}_{bb}",
                                         tag=f"pss{bb}", bufs=1)
                    for c in range(8):
                        for m in range(2):
                            nc.tensor.matmul(
                                pss[:, c, :],
                                kt[:, bb, m, 128 * c:128 * c + 128],
                                qbd[:, m, 32 * b:32 * b + 32],
                                start=(m == 0), stop=(m == 1))
                    nc.scalar.activation(attnE[:, :, bb, :], pss,
                                         AF.Exp, bias=nbias_c, scale=0.125)
                for bb in (3, 0, 1, 2):
                    psum_b = pr[0:1, 256 + 32 * bb:256 + 32 * bb + 32]
                    for c in range(8):
                        nc.tensor.matmul(psum_b, ones_c[:, 0:1],
                                         attnE[:, c, bb, :],
                                         start=(c == 0), stop=(c == 7))
                if r + 5 < N_ROUNDS:
                    t5 = fetch_cache(r + 5)
                    fetch_vnew(r + 5, t5[1])
                    tiles[r + 5] = t5
                    fill_knew(r + 5, t5[0])
                # ---- softmax denominators (broadcast happens in av_part1)
                recip = small.tile([1, 128], F32, name=f"recip_{r}",
                                   tag="recip", bufs=2)
                nc.vector.reciprocal(recip, pr[0:1, 256:384])
                if prev is not None:
                    wo_round(prev[0])
                prev = (r, vt, attnE, pav, psbc, recip)
            # rounds 0-6 output DMA first: its transfer fills the
            # post-stream idle window while round 7's V-phase finishes
            nc.sync.dma_start(
                outT[:, 0:7, :, :],
                osbT[:, 0:7, :].rearrange("p r (ch t) -> p r ch t", t=32))
            av_part1(*prev)
            evac_chain(prev[0], prev[3], prev[4], prev[5])
            wo_round(prev[0])

    nc.finalize()
    return nc


_SIGMA = None


def _sigma():
    # sigma(j) = virtual key index at score column j
    global _SIGMA
    if _SIGMA is None:
        j = np.arange(S)
        _SIGMA = 8 * (j % 128) + j // 128
    return _SIGMA


F16_NP = np.float16
FP8_NP = ml_dtypes.float8_e3m4


def _prep_core(c, x_flat_T, cache_keys, cache_values,
               Wq, bq, Wk, bk, Wv, bv, Wo, bo=None, cfg=CFG):
    hs = slice(HC * c, HC * c + HC)
    qs = slice(QD * c, QD * c + QD)

    def perm_rows(W):
        # rows ordered (m, h, j): row 32h + j of tile m = W[64h + 32m + j]
        Ws = W[qs].reshape(HC, 2, 32, -1)              # [h, m, j, d]
        return Ws.transpose(1, 0, 2, 3).reshape(QD, -1)  # [(m,h,j), d]

    wq_p = perm_rows(Wq)
    wk_p = perm_rows(Wk)
    bq_p = np.ascontiguousarray(perm_rows(bq[:, None])[:, 0])
    bk_p = np.ascontiguousarray(perm_rows(bk[:, None])[:, 0])

    def as_tiles(WT):  # [D, 256] -> [128, 16, 256]
        return np.ascontiguousarray(
            WT.reshape(16, 128, QD).transpose(1, 0, 2)).astype(F16_NP)

    wqT = as_tiles(np.ascontiguousarray(wq_p.T))
    wkT = (as_tiles(np.ascontiguousarray(wk_p.T)).astype(np.float32)
           * WKV_SCALE).astype(FP8_NP)
    wvT = (as_tiles(np.ascontiguousarray(Wv[qs].T)).astype(np.float32)
           * WKV_SCALE).astype(FP8_NP)
    woT = np.ascontiguousarray(
        Wo[:, qs].T.reshape(2, 128, D).transpose(1, 0, 2)).astype(F16_NP)

    # kT[p=(h,j), b, m, jcol]: keys sigma-permuted; zero at new-key columns
    ck = cache_keys[:, hs]                        # [B, 4, 1016, 64]
    kmat = ck.reshape(B, HC, S_CACHE, 2, 32).transpose(0, 3, 1, 4, 2)  # b m h j s
    kmat = np.ascontiguousarray(kmat.reshape(B, 2, 128, S_CACHE))
    kT = np.zeros((B, 2, 128, S), dtype=np.float32)
    sig = _sigma()
    valid = sig < S_CACHE
    kT[:, :, :, valid] = kmat[:, :, :, sig[valid]]
    kT = np.ascontiguousarray(kT.transpose(2, 0, 1, 3)).astype(FP8_NP)

    # v[p, b, i, h, d] = cache_values[b, h, 8p + i, d]
    cv = cache_values[:, hs].reshape(B, HC, NP, 8, DH)
    v_pack = np.ascontiguousarray(cv.transpose(2, 0, 3, 1, 4)).astype(FP8_NP)

    return {
        "xT": x_flat_T.astype(F16_NP),
        "wqT": wqT, "wkT": wkT, "wvT": wvT, "woT": woT,
        "bq": bq_p, "bk": bk_p,
        "bv": np.ascontiguousarray(bv[qs]),
        "kT": kT,
        "v": v_pack,
    }


def merge_outputs(outT_arr):
    """Un-transpose a core's partial: outT[p, r, ch, t] -> [256, 2048]."""
    tl = np.asarray(outT_arr, dtype=np.float32)  # [128, 8, 16, 32]
    return tl.transpose(1, 3, 2, 0).reshape(TOK, D)


_NC_CACHE = {}


def kernel(x, cache_keys, cache_values, Wq, bq, Wk, bk, Wv, bv, Wo, bo):
    x = np.asarray(x, dtype=np.float32)
    cache_keys = np.asarray(cache_keys, dtype=np.float32)
    cache_values = np.asarray(cache_values, dtype=np.float32)
    Wq, Wk, Wv, Wo = (np.asarray(w, dtype=np.float32) for w in (Wq, Wk, Wv, Wo))
    bq, bk, bv, bo = (np.asarray(b_, dtype=np.float32) for b_ in (bq, bk, bv, bo))

    x_flat_T = np.ascontiguousarray(
        x.reshape(TOK, D).T.reshape(16, 128, TOK).transpose(1, 0, 2))  # [128,16,256]

    in_maps = [
        _prep_core(c, x_flat_T, cache_keys, cache_values,
                   Wq, bq, Wk, bk, Wv, bv, Wo)
        for c in range(N_CORES)
    ]

    key = tuple(sorted(CFG.items()))
    if key not in _NC_CACHE:
        _NC_CACHE[key] = build_nc(CFG)
    nc = _NC_CACHE[key]

    res = bass_utils.run_bass_kernel_spmd(nc, in_maps, core_ids=list(range(N_CORES)))
    out = np.zeros((TOK, D), dtype=np.float32)
    for r in res.results:
        out += merge_outputs(r["outT"])
    out += bo
    return out.reshape(B, T, D)
